# revision 51
# baseline (speedup 1.0000x reference)
"""Trainium2 Bass kernel for nn_MultiHeadAttention_77249281786483.

GQA multi-head attention (B=2, S=2048, D=2048, H=32, KVH=8, HD=64) with
interleaved RoPE and causal masking, distributed over 8 NeuronCores with
zero collectives:

  - core c -> batch b=c//4, stripe j=c%4, owning q-tiles {j, 4+j, 8+j, 12+j}
    (4 x 128 query rows).  Striped assignment makes the causal attention
    work identical on every core, so a single SPMD graph serves all cores
    and all per-core variation lives in the input data.
  - every core computes the full K/V for its batch; Q only for its rows.
  - bf16 matmuls with fp32 PSUM accumulation; scores are built transposed
    [k, q]; V is ones-augmented so the ctx matmul also emits the softmax
    denominator pre-broadcast to 64 partitions at no extra PE cost.
  - RoPE is applied in the transposed projection layout on DVE+GPSIMD.
  - the attention phase is a single flat pipeline over (m, group) steps:
    exp on the ACT engine is the rate limiter, so ctx matmuls lag the
    score matmuls by L flat steps (crossing m boundaries), and all
    remaining projection work (Q chunks 8-15, second-half K) is spread
    across the steps as PE filler.  Short causal key-tiles (8,9), (10,11)
    and (12..15) are packed into single [128,1024] score tiles so the
    exp/mask work runs in 11 ACT calls per m instead of 16.
  - a warm-up burst of dummy matmuls at t=0 covers the DMA ring startup
    latency and flips the PE HAM clock gate to full rate before real work.
  - output projection runs qi-outer so each 128-row output block's
    PSUM->SBUF copy (which also folds in the bias) and DMA overlap the
    next block's matmuls; W_o is prefetched during late attention.
"""

from contextlib import ExitStack

import numpy as np
import ml_dtypes

import concourse.bass as bass  # noqa: F401  (bass types via bacc)
import concourse.mybir as mybir
import concourse.tile as tile
from concourse import bacc
from concourse.bass_utils import run_bass_kernel_spmd

BF16 = mybir.dt.bfloat16
F32 = mybir.dt.float32
BFNP = ml_dtypes.bfloat16

B, S, D = 2, 2048, 2048
H, KVH, HD = 32, 8, 64
N_CORES = 8

MULT = mybir.AluOpType.mult
ADD = mybir.AluOpType.add
EXP = mybir.ActivationFunctionType.Exp

_BUILD_CACHE = {}

# attention key-tiles grouped per exp/mask call: the short causal tiles
# pack 2 or 4 key-tiles into one [128,1024] PSUM score tile
GROUPS = [[0], [1], [2], [3], [4], [5], [6], [7],
          [8, 9], [10, 11], [12, 13, 14, 15]]
NG = len(GROUPS)          # 11 steps per m
L = 6                     # flat-step lag between scores/exp and ctx
Q_PREFIX = 10             # Q chunks built before V/K (early phase is
                          # DMA-ramp-bound: Q work is data-light filler)
WARM_MM = 32              # dummy matmuls at t=0 (DMA latency + HAM warm)

# merged Q-projection contraction chunks per in-m step (16 dc over 11)
DC_FULL = [[0, 1], [2, 3], [4, 5], [6, 7], [8, 9],
           [10], [11], [12], [13], [14], [15]]
# half chunk: 8 dc over 11 steps
DC_HALF = [[0], [1], [2], [3], [4], [5], [6], [7], [], [], []]


def _build_nc_v5():
    nc = bacc.Bacc("TRN2", target_bir_lowering=False, debug=False,
                   num_devices=N_CORES)

    xT_d = nc.dram_tensor("xT", [128, 16, 2048], BF16,
                          kind="ExternalInput").ap()
    xqT_d = nc.dram_tensor("xqT", [128, 16, 512], BF16,
                           kind="ExternalInput").ap()
    wq_d = nc.dram_tensor("wq", [128, 16, 16, 128], BF16,
                          kind="ExternalInput").ap()
    wk_d = nc.dram_tensor("wk", [128, 16, 512], BF16,
                          kind="ExternalInput").ap()
    wv_d = nc.dram_tensor("wv", [128, 16, 512], BF16,
                          kind="ExternalInput").ap()
    wo_d = nc.dram_tensor("wo", [128, 4, 4, 4, 512], BF16,
                          kind="ExternalInput").ap()
    cq_d = nc.dram_tensor("cq", [128, 512], BF16, kind="ExternalInput").ap()
    sq_d = nc.dram_tensor("sq", [128, 512], BF16, kind="ExternalInput").ap()
    ck_d = nc.dram_tensor("ck", [128, 2048], BF16, kind="ExternalInput").ap()
    sk_d = nc.dram_tensor("sk", [128, 2048], BF16, kind="ExternalInput").ap()
    keep_d = nc.dram_tensor("keep", [128, 1024], BF16,
                            kind="ExternalInput").ap()
    out_d = nc.dram_tensor("out", [512, D], BF16, kind="ExternalOutput").ap()

    with ExitStack() as st:
        tc = st.enter_context(tile.TileContext(nc))

        # ---- warm-up: dummy matmuls keep the PE busy through the DMA
        # ring startup and flip the HAM clock gate to 8/8 before real
        # matmuls arrive ----
        warm_st = ExitStack()
        pwarm = warm_st.enter_context(tc.tile_pool(name="pwarm", bufs=1))
        pswm = warm_st.enter_context(tc.tile_pool(name="pswm", bufs=1,
                                                  space="PSUM"))
        wt = pwarm.tile([128, 512], BF16, tag="wt")
        nc.vector.memset(wt[:], 0.0)
        wps = pswm.tile([128, 512], F32, tag="wps")
        for i in range(WARM_MM):
            nc.tensor.matmul(wps[:], wt[:, 0:128], wt[:],
                             start=(i == 0), stop=(i == WARM_MM - 1))
        warm_st.close()

        pers = st.enter_context(tc.tile_pool(name="pers", bufs=1))
        qt = pers.tile([128, 16, 512], BF16, tag="qt")
        ktq = pers.tile([128, 4, 2048], BF16, tag="ktq")
        vv = pers.tile([128, 16, 8, 128], BF16, tag="vv")
        ctxT = pers.tile([128, 16, 512], BF16, tag="ctxT")
        keep = pers.tile([128, 2, 4, 128], BF16, tag="keep")

        nc.vector.memset(vv[:, :, :, 64:128], 1.0)

        def ktq_wo(cq):
            # es=0 W_o quarter staged into a dead ktq kv-slot
            return ktq[:, cq, :].rearrange("p (c4 n) -> p c4 n", c4=4)

        # Q-side pools live from the prefix through the merged section.
        paQ = st.enter_context(tc.tile_pool(name="paQ", bufs=1))
        paw = st.enter_context(tc.tile_pool(name="paw", bufs=5))
        pscr = st.enter_context(tc.tile_pool(name="pscr", bufs=2))
        psQ2 = st.enter_context(tc.tile_pool(name="psQ2", bufs=1,
                                             space="PSUM"))
        psQpre_st = ExitStack()
        psQpre = psQpre_st.enter_context(tc.tile_pool(name="psQpre", bufs=2,
                                                      space="PSUM"))
        xqT = paQ.tile([128, 16, 512], BF16, tag="xqT")
        cqt = paQ.tile([128, 512], BF16, tag="cqt")
        sqt = paQ.tile([128, 512], BF16, tag="sqt")

        def q_start(c, pool):
            wqs = paw.tile([128, 16, 128], BF16, tag="wq", name=f"wqs{c}")
            nc.sync.dma_start(out=wqs[:], in_=wq_d[:, c, :, :])
            ps = pool.tile([128, 512], F32, tag="qmm", name=f"psq{c}")
            return wqs, ps

        def q_mm(st_, c, dc):
            wqs, ps = st_
            nc.tensor.matmul(ps[:], wqs[:, dc, :], xqT[:, dc, :],
                             start=(dc == 0), stop=(dc == 15))

        def rope_dve(ps_in, ctab, stab, out_ap, name):
            # PE-free RoPE: bf16 staging + 4 cross-quadrant copies (on the
            # otherwise idle GPSIMD) implement the p <-> p^32 partition
            # swap; rotation arithmetic in bf16 on the DVE.
            raw = pscr.tile([128, 512], BF16, tag="raw", name=f"raw{name}")
            nc.vector.tensor_copy(out=raw[:], in_=ps_in)
            swp = pscr.tile([128, 512], BF16, tag="swp", name=f"swp{name}")
            for blk in range(4):
                srcb = (blk ^ 1) * 32
                nc.vector.tensor_copy(out=swp[32 * blk:32 * blk + 32, :],
                                      in_=raw[srcb:srcb + 32, :])
            t1 = pscr.tile([128, 512], BF16, tag="t1", name=f"t1{name}")
            t2 = pscr.tile([128, 512], BF16, tag="t2", name=f"t2{name}")
            nc.vector.tensor_tensor(out=t1[:], in0=raw[:], in1=ctab, op=MULT)
            nc.vector.tensor_tensor(out=t2[:], in0=swp[:], in1=stab, op=MULT)
            nc.vector.tensor_tensor(out=out_ap, in0=t1[:], in1=t2[:], op=ADD)

        def q_finish(st_, c):
            for dc in range(16):
                q_mm(st_, c, dc)
            rope_dve(st_[1][:], cqt[:], sqt[:], qt[:, c, :], f"q{c}")

        # The bulk of the input streams on the sync HWDGE ring with the
        # Q-critical prefix at its head; xT's upper chunks ride the
        # otherwise-idle scalar ring in parallel so the V projections
        # (which contract over all 16 chunks) start without a DMA stall.
        def q_start_scalar(c, pool):
            # early chunks ride the scalar ring, parallel to the sync ring
            wqs = paw.tile([128, 16, 128], BF16, tag="wq", name=f"wqs{c}")
            nc.scalar.dma_start(out=wqs[:], in_=wq_d[:, c, :, :])
            ps = pool.tile([128, 512], F32, tag="qmm", name=f"psq{c}")
            return wqs, ps

        qsts = [q_start(0, psQpre)]
        nc.sync.dma_start(out=xqT[:, 0:8, :], in_=xqT_d[:, 0:8, :])
        nc.scalar.dma_start(out=xqT[:, 8:16, :], in_=xqT_d[:, 8:16, :])
        qsts.append(q_start_scalar(1, psQpre))
        qsts.append(q_start_scalar(2, psQpre))
        qsts.append(q_start_scalar(3, psQpre))
        nc.sync.dma_start(out=cqt[:], in_=cq_d)
        nc.sync.dma_start(out=sqt[:], in_=sq_d)
        for c in range(4, Q_PREFIX):
            qsts.append(q_start(c, psQpre))
        for c in range(Q_PREFIX):
            q_finish(qsts[c], c)
        psQpre_st.close()

        # ---- V/K section ----
        # xT/wk/rope tables stay resident through attention (stVK) so the
        # second-half K projections can overlap the attention phase; wv
        # closes right after the V projections to free SBUF.
        stVK = ExitStack()
        paVK = stVK.enter_context(tc.tile_pool(name="paVK", bufs=1))
        psVK0 = ExitStack()
        psVK = psVK0.enter_context(tc.tile_pool(name="psVK", bufs=3,
                                                space="PSUM"))
        xT = paVK.tile([128, 16, 1024], BF16, tag="xT")
        wk = paVK.tile([128, 16, 512], BF16, tag="wk")
        ckt = paVK.tile([128, 2048], BF16, tag="ckt")
        skt = paVK.tile([128, 2048], BF16, tag="skt")

        def v_proj(mt):
            ps = psVK.tile([128, 512], F32, tag="mm", name=f"psv{mt}")
            mloc = 128 * (mt % 8)
            for dc in range(16):
                nc.tensor.matmul(
                    ps[:], xT[:, dc, mloc:mloc + 128], wv[:, dc, :],
                    start=(dc == 0), stop=(dc == 15))
            nc.vector.tensor_copy(
                out=vv[:, mt, :, 0:64],
                in_=ps[:].rearrange("p (g d) -> p g d", g=8))

        def k_proj(cc, ms, pool):
            ps = pool.tile([128, 512], F32, tag="mm", name=f"psk{cc}_{ms}")
            msl = 512 * (ms % 2)
            for dc in range(16):
                nc.tensor.matmul(
                    ps[:], wk[:, dc, 128 * cc:128 * (cc + 1)],
                    xT[:, dc, msl:msl + 512],
                    start=(dc == 0), stop=(dc == 15))
            rope_dve(ps[:], ckt[:, 512 * ms:512 * (ms + 1)],
                     skt[:, 512 * ms:512 * (ms + 1)],
                     ktq[:, cc, 512 * ms:512 * (ms + 1)],
                     f"k{cc}_{ms}")

        with tc.tile_pool(name="pwv", bufs=1) as pwv:
            wv = pwv.tile([128, 16, 512], BF16, tag="wv")
            nc.scalar.dma_start(out=xT[:, 4:16, :],
                                in_=xT_d[:, 4:16, 0:1024])
            nc.scalar.dma_start(out=wv[:], in_=wv_d)
            nc.sync.dma_start(out=xT[:, 0:4, :], in_=xT_d[:, 0:4, 0:1024])
            nc.sync.dma_start(out=ckt[:], in_=ck_d)
            nc.sync.dma_start(out=skt[:], in_=sk_d)
            nc.sync.dma_start(out=wk[:], in_=wk_d)
            nc.sync.dma_start(
                out=keep[:], in_=keep_d.rearrange("p (s k n) -> p s k n",
                                                  s=2, k=4))

            for mt in range(8):
                v_proj(mt)
            for cc in range(4):
                for ms in (0, 1):
                    k_proj(cc, ms, psVK)
            for c in range(16):
                nc.scalar.dma_start(out=xT[:, c, :],
                                    in_=xT_d[:, c, 1024:2048])
            for mt in range(8, 16):
                v_proj(mt)
        psVK0.close()

        # ---- merged section: one flat pipeline over (m, group) steps.
        # exp on ACT is the rate limiter; ctx matmuls lag the score
        # matmuls by L flat steps so the PE never waits on ACT, and the
        # remaining Q chunks / second-half K projections fill PE slack ----
        stq = ExitStack()
        pb = stq.enter_context(tc.tile_pool(name="pb", bufs=8))
        pbn = stq.enter_context(tc.tile_pool(name="pbn", bufs=1))
        psS = stq.enter_context(tc.tile_pool(name="psS", bufs=2,
                                             space="PSUM"))
        psC = stq.enter_context(tc.tile_pool(name="psC", bufs=2,
                                             space="PSUM"))
        psK = stq.enter_context(tc.tile_pool(name="psK", bufs=1,
                                             space="PSUM"))

        # kvc=0 second-half keys must be ready at the first attention m
        for ms in (2, 3):
            k_proj(0, ms, psK)

        deferred = []
        qstate = {}
        kstate = {}

        def fq_start(c):
            qstate["cur"] = (c, q_start(c, psQ2))

        def fq_mm(dc):
            c, st_ = qstate["cur"]
            q_mm(st_, c, dc)

        def fq_rope(defer=True):
            c, st_ = qstate["cur"]
            raw = pscr.tile([128, 512], BF16, tag="raw", name=f"rawq{c}")
            nc.vector.tensor_copy(out=raw[:], in_=st_[1][:])

            def rope_rest(c=c, raw=raw):
                swp = pscr.tile([128, 512], BF16, tag="swp", name=f"swpq{c}")
                for blk in range(4):
                    srcb = (blk ^ 1) * 32
                    nc.vector.tensor_copy(
                        out=swp[32 * blk:32 * blk + 32, :],
                        in_=raw[srcb:srcb + 32, :])
                t1 = pscr.tile([128, 512], BF16, tag="t1", name=f"t1q{c}")
                t2 = pscr.tile([128, 512], BF16, tag="t2", name=f"t2q{c}")
                nc.vector.tensor_tensor(out=t1[:], in0=raw[:], in1=cqt[:],
                                        op=MULT)
                nc.vector.tensor_tensor(out=t2[:], in0=swp[:], in1=sqt[:],
                                        op=MULT)
                nc.vector.tensor_tensor(out=qt[:, c, :], in0=t1[:],
                                        in1=t2[:], op=ADD)

            deferred.append(rope_rest)

        def fk_start(cc, ms):
            ps = psK.tile([128, 512], F32, tag="mm", name=f"psk{cc}_{ms}")
            kstate["cur"] = (cc, ms, ps)

        def fk_mm(dc):
            cc, ms, ps = kstate["cur"]
            msl = 512 * (ms % 2)
            nc.tensor.matmul(
                ps[:], wk[:, dc, 128 * cc:128 * (cc + 1)],
                xT[:, dc, msl:msl + 512],
                start=(dc == 0), stop=(dc == 15))

        def fk_finish():
            cc, ms, ps = kstate["cur"]
            rope_dve(ps[:], ckt[:, 512 * ms:512 * (ms + 1)],
                     skt[:, 512 * ms:512 * (ms + 1)],
                     ktq[:, cc, 512 * ms:512 * (ms + 1)],
                     f"k{cc}_{ms}")

        # per-(m, g) filler schedule: Q chunks 8..15 and second-half K
        # projections for kvc 1..3, spread to finish just before use
        FILL = [[[] for _ in range(NG)] for _ in range(16)]

        def sched_qfull(m, c):
            FILL[m][0].append(("qs", c))
            for g in range(NG):
                for dc in DC_FULL[g]:
                    FILL[m][g].append(("q", dc))
            FILL[m][NG - 1].append(("qr",))

        def sched_qhalf(m, c, half):
            if half == 0:
                FILL[m][0].append(("qs", c))
            for g in range(NG):
                for dc in DC_HALF[g]:
                    FILL[m][g].append(("q", 8 * half + dc))
            if half == 1:
                FILL[m][7].append(("qr",))

        def sched_k(m, cc, ms):
            FILL[m][0].append(("ks", cc, ms))
            for g in range(NG):
                for dc in DC_FULL[g]:
                    FILL[m][g].append(("k", dc))
            FILL[m][NG - 1].append(("kf",))

        sched_qfull(0, 10)
        sched_qfull(1, 11)
        sched_k(2, 1, 2)
        sched_k(3, 1, 3)
        sched_qhalf(4, 12, 0)
        sched_qhalf(5, 12, 1)
        sched_k(6, 2, 2)
        sched_k(7, 2, 3)
        sched_qhalf(8, 13, 0)
        sched_qhalf(9, 13, 1)
        sched_k(10, 3, 2)
        sched_k(11, 3, 3)
        sched_qhalf(10, 14, 0)
        sched_qhalf(11, 14, 1)
        sched_qhalf(12, 15, 0)
        sched_qhalf(13, 15, 1)

        # W_o es=0 quarters stream into dead ktq kv-slots as attention
        # retires them; the m=14/15 steps (no projection filler left) run
        # partial output-projection accumulations in the freed psK/psQ2
        # banks, which also bridges the final-norm drain before the O phase
        FILL[4][0].append(("wo2kt", 0))
        FILL[8][0].append(("wo2kt", 1))
        FILL[12][0].append(("wo2kt", 2))
        FILL[14][0].append(("os", 0))
        FILL[15][0].append(("os", 1))
        for g in range(8):
            FILL[14][g].append(("o", 0, g))
            FILL[15][g].append(("o", 1, g))

        ostate = {}

        def o_mm(qi, c):
            nc.tensor.matmul(
                ostate[qi][:], ctxT[:, c, 128 * qi:128 * (qi + 1)],
                ktq_wo(c // 4)[:, c % 4, :],
                start=(c == 0), stop=(c == 15))

        def run_fill(op):
            if op[0] == "qs":
                fq_start(op[1])
            elif op[0] == "q":
                fq_mm(op[1])
            elif op[0] == "qr":
                fq_rope()
            elif op[0] == "ks":
                fk_start(op[1], op[2])
            elif op[0] == "k":
                fk_mm(op[1])
            elif op[0] == "kf":
                fk_finish()
            elif op[0] == "wo2kt":
                cq = op[1]
                nc.sync.dma_start(out=ktq_wo(cq), in_=wo_d[:, 0, cq, :, :])
            elif op[0] == "os":
                qi = op[1]
                pool, tag = (psK, "mm") if qi == 0 else (psQ2, "qmm")
                ostate[qi] = pool.tile([128, 512], F32, tag=tag,
                                       name=f"opart{qi}")
            elif op[0] == "o":
                o_mm(op[1], op[2])

        def norm(m, s, cxs):
            sums = pbn.tile([64, 512], F32, tag="sums", name=f"sums{m}_{s}")
            nc.vector.tensor_copy(out=sums[:], in_=cxs[s][64:128, :])
            rec = pbn.tile([64, 512], F32, tag="rec", name=f"rec{m}_{s}")
            # seed-only reciprocal (~12 bits) is plenty under the 2e-2
            # budget and halves the DVE cost of every softmax normalize
            nc.vector.reciprocal_approx_fast(out=rec[:], in_=sums[:])
            nc.vector.tensor_tensor(
                out=ctxT[64 * s:64 * (s + 1), m, :],
                in0=cxs[s][0:64, :], in1=rec[:], op=MULT)

        NSTEP = 16 * NG
        pend = {}
        cxs_by_m = {}
        for i in range(NSTEP + L):
            if i < NSTEP:
                m, g = divmod(i, NG)
                kvc = m // 4
                group = GROUPS[g]
                glen = len(group)
                q0 = 128 * (group[0] // 4)
                nq = 512 - q0
                sc = psS.tile([128, 1024], F32, tag="sc", name=f"sc{i}")
                # head-slot matmuls are K=64 row-tiles issued adjacent on
                # the PE queue so they run concurrently in the array; the
                # (s, g, n) layout keeps each concurrent pair's outputs in
                # DIFFERENT PSUM banks (slot 0 -> bank 0, slot 1 -> bank 1)
                stride = 512 // glen
                with tc.high_priority(offset=24):
                    for ktl, kt in enumerate(group):
                        for s2 in range(2):
                            off = 512 * s2 + ktl * stride
                            nc.tensor.matmul(
                                sc[0:128, off:off + nq],
                                ktq[64 * s2:64 * (s2 + 1), kvc,
                                    128 * kt:128 * (kt + 1)],
                                qt[64 * s2:64 * (s2 + 1), m, q0:512],
                                start=True, stop=True)
                at = pb.tile([128, 1024], BF16, tag="at", name=f"at{i}")
                pend[i] = at
                scv = sc[:].rearrange("p (s g n) -> p s g n", s=2, g=glen)
                atv = at[:].rearrange("p (s g n) -> p s g n", s=2, g=glen)
                nc.scalar.activation(out=atv[:, :, :, 0:nq],
                                     in_=scv[:, :, :, 0:nq],
                                     func=EXP, scale=0.125)
                k0 = group[0] % 4
                nc.vector.tensor_tensor(
                    out=atv[:, :, :, 0:128], in0=atv[:, :, :, 0:128],
                    in1=keep[:, :, k0:k0 + glen, :], op=MULT)
                for op in FILL[m][g]:
                    run_fill(op)
                if deferred:
                    deferred.pop(0)()
            if i >= L:
                i2 = i - L
                m2, g2 = divmod(i2, NG)
                kvc2 = m2 // 4
                if g2 == 0:
                    cxs_by_m[m2] = [
                        psC.tile([128, 512], F32, tag="ctx",
                                 name=f"cx{m2}_{s2}")
                        for s2 in range(2)]
                group = GROUPS[g2]
                glen = len(group)
                q0 = 128 * (group[0] // 4)
                nq = 512 - q0
                at = pend.pop(i2)
                atv = at[:].rearrange("p (s g n) -> p s g n", s=2, g=glen)
                cxs = cxs_by_m[m2]
                for ktl, kt in enumerate(group):
                    for s2 in range(2):
                        nc.tensor.matmul(
                            cxs[s2][0:128, q0:512],
                            vv[:, kt, 2 * kvc2 + s2, :],
                            atv[:, s2, ktl, 0:nq],
                            start=(kt == 0), stop=(kt == 15))
                if g2 == NG - 1:
                    deferred.append(
                        lambda m2=m2: norm(m2, 0, cxs_by_m[m2]))
                    deferred.append(
                        lambda m2=m2: norm(m2, 1, cxs_by_m[m2]))

        # last W_o es=0 quarter into the just-retired ktq slot 3, then
        # finish the two partial output blocks while the final norms drain
        nc.sync.dma_start(out=ktq_wo(3), in_=wo_d[:, 0, 3, :, :])
        for qi in (0, 1):
            for c in range(8, 14):
                o_mm(qi, c)
        while deferred:
            deferred.pop(0)()
        for qi in (0, 1):
            o_mm(qi, 14)
            o_mm(qi, 15)
        for qi in (0, 1):
            ogp = pb.tile([128, 512], BF16, tag="ogp", name=f"ogp{qi}",
                          bufs=2)
            nc.vector.tensor_copy(out=ogp[:], in_=ostate[qi][:])
            nc.scalar.dma_start(out=out_d[128 * qi:128 * (qi + 1), 0:512],
                                in_=ogp[:])
        stq.close()   # release merged-section PSUM/SBUF pools
        stVK.close()  # release xT/wk/rope tables

        # ---- output projection: qi-outer so each 128-row block's copy
        # and DMA overlap the next block's matmuls; b_o is added on host ----
        pcw = st.enter_context(tc.tile_pool(name="pcw", bufs=8))
        pco = st.enter_context(tc.tile_pool(name="pco", bufs=2))
        psO = st.enter_context(tc.tile_pool(name="psO", bufs=7, space="PSUM"))
        woq = {}

        def woe_dma(es, cq):
            # weights keep the sync ring to themselves; outputs go scalar
            w = pcw.tile([128, 4, 512], BF16, tag="wo", name=f"wo{es}_{cq}")
            nc.sync.dma_start(out=w[:], in_=wo_d[:, es, cq, :, :])
            woq[(es, cq)] = w[:]

        for es in range(4):
            for cq in range(4):
                if es > 0 and (es, cq) not in woq:
                    woe_dma(es, cq)
            qis = (2, 3) if es == 0 else (0, 1, 2, 3)
            for idx, qi in enumerate(qis):
                if es < 3 and idx == 0:
                    woe_dma(es + 1, 0)
                    woe_dma(es + 1, 1)
                if es < 3 and idx == len(qis) - 2:
                    woe_dma(es + 1, 2)
                    woe_dma(es + 1, 3)
                po = psO.tile([128, 512], F32, tag="out",
                              name=f"po{es}_{qi}")
                for cq in range(4):
                    w = ktq_wo(cq) if es == 0 else woq[(es, cq)]
                    for c in range(4 * cq, 4 * cq + 4):
                        nc.tensor.matmul(
                            po[:], ctxT[:, c, 128 * qi:128 * (qi + 1)],
                            w[:, c % 4, :],
                            start=(c == 0), stop=(c == 15))
                og = pco.tile([128, 512], BF16, tag="og", name=f"og{es}_{qi}")
                nc.vector.tensor_copy(out=og[:], in_=po[:])
                nc.scalar.dma_start(
                    out=out_d[128 * qi:128 * (qi + 1),
                              512 * es:512 * (es + 1)],
                    in_=og[:])
            for cq in range(4):
                woq.pop((es, cq), None)

    nc.compile()
    return nc


def _get_nc():
    if "nc" not in _BUILD_CACHE:
        _BUILD_CACHE["nc"] = _build_nc_v5()
    return _BUILD_CACHE["nc"]


def _build_perms():
    r = np.arange(2048)
    m, rr = r // 128, r % 128
    s, half, jd = rr // 64, (rr % 64) // 32, rr % 32
    h = 8 * (m // 4) + 4 * s + (m % 4)
    qperm = 64 * h + 2 * jd + half
    woperm = 64 * h + (rr % 64)
    rk = np.arange(512)
    ck, rrk = rk // 128, rk % 128
    sk_, halfk, jdk = rrk // 64, (rrk % 64) // 32, rrk % 32
    kperm = 64 * (2 * ck + sk_) + 2 * jdk + halfk
    return qperm, kperm, woperm


def _rope_tables(fc, fs, positions):
    p = np.arange(128)
    jd = p % 32
    sign = np.where((p % 64) < 32, -1.0, 1.0).astype(np.float32)
    C2 = np.ascontiguousarray(fc[positions][:, jd].T.astype(BFNP))
    S2 = np.ascontiguousarray(
        (fs[positions][:, jd].T * sign[:, None]).astype(BFNP))
    return C2, S2


def prepare_in_maps(inputs):
    x = np.asarray(inputs["x"], np.float32)
    Wq = np.asarray(inputs["W_q"], np.float32)
    Wk = np.asarray(inputs["W_k"], np.float32)
    Wv = np.asarray(inputs["W_v"], np.float32)
    Wo = np.asarray(inputs["W_o"], np.float32)
    bo = np.asarray(inputs["b_o"], np.float32)
    fc = np.asarray(inputs["freqs_cos"], np.float32)
    fs = np.asarray(inputs["freqs_sin"], np.float32)

    qperm, kperm, woperm = _build_perms()
    # [p, c, dc, n]: chunk c's weights contiguous per partition (4KB lines)
    wq_host = np.ascontiguousarray(
        Wq[:, qperm].astype(BFNP).reshape(16, 128, 16, 128)
        .transpose(1, 2, 0, 3))
    wk_host = np.ascontiguousarray(
        Wk[:, kperm].astype(BFNP).reshape(16, 128, 512).transpose(1, 0, 2))
    wv_host = np.ascontiguousarray(
        Wv.astype(BFNP).reshape(16, 128, 512).transpose(1, 0, 2))
    # [p, es, cq, c4, n]
    wo_host = np.ascontiguousarray(
        Wo[woperm, :].astype(BFNP).reshape(4, 4, 128, 4, 512)
        .transpose(2, 3, 0, 1, 4))
    ck_t, sk_t = _rope_tables(fc, fs, np.arange(2048))

    xT_b = []
    for b in range(B):
        xT_b.append(np.ascontiguousarray(
            x[b].T.astype(BFNP).reshape(16, 128, 2048).transpose(1, 0, 2)))

    in_maps = []
    for core in range(N_CORES):
        b, j = core // 4, core % 4
        qpos = (np.arange(4)[:, None] * 512 + 128 * j
                + np.arange(128)[None, :]).reshape(-1)
        xqT = np.ascontiguousarray(
            x[b][qpos].T.astype(BFNP).reshape(16, 128, 512)
            .transpose(1, 0, 2))
        cq_t, sq_t = _rope_tables(fc, fs, qpos)
        kp = np.arange(128)[:, None]
        qf = np.arange(128)[None, :]
        keep4 = np.stack(
            [((128 * ktp + kp - 128 * j) <= qf) for ktp in range(4)],
            axis=1)  # [128, 4, 128]
        # s-major: [128, s, ktp, n] so both head slots share one mask view
        keep = np.repeat(keep4[:, None, :, :], 2, axis=1) \
            .reshape(128, 1024).astype(BFNP)
        in_maps.append({
            "xT": xT_b[b], "xqT": xqT, "wq": wq_host, "wk": wk_host,
            "wv": wv_host, "wo": wo_host,
            "cq": cq_t, "sq": sq_t, "ck": ck_t, "sk": sk_t,
            "keep": np.ascontiguousarray(keep),
        })
    return in_maps


def kernel(**inputs):
    nc = _get_nc()
    in_maps = prepare_in_maps(inputs)
    res = run_bass_kernel_spmd(nc, in_maps, core_ids=list(range(N_CORES)))
    bo = np.asarray(inputs["b_o"], np.float32)
    out = np.zeros((B, S, D), np.float32)
    for core in range(N_CORES):
        b, j = core // 4, core % 4
        qpos = (np.arange(4)[:, None] * 512 + 128 * j
                + np.arange(128)[None, :]).reshape(-1)
        out[b][qpos] = np.asarray(res.results[core]["out"],
                                  dtype=np.float32) + bo
    return out
